# revision 2
# baseline (speedup 1.0000x reference)
"""Trainium2 Bass kernel v2 for nn_Attention_18760417149505.

Reference (per problem):
  q/k/v = (x @ W.T + b).reshape(B, H, S, dk)      # flat reshape, NOT head-split
  scores = q @ k.T ; t = (scores*SCALE) @ v ; attn = softmax(t, axis=-1)
  out = ((attn.reshape(B,S,D) @ Wo.T + bo) @ Wf.T + bf)

Softmax comes AFTER both score matmuls, so per (batch, head) only the 64x64
Gram matrix G = SCALE * k.T @ v is needed:  t = q @ G.

v2 design (vs the fp32r baseline):
  * fp16 on the whole matmul path (1 cycle/row at ANY free size, 2-byte DMA).
    Empirically rel_err ~4.3e-3 vs the 2e-2 gate; bf16 fails (3.3e-2) because
    the pre-softmax logits (std ~45) make softmax near-argmax.
  * Wf@Wo folded into one weight on the host: one 768x768 projection less.
  * Q/K/V all feature-major with per-partition ACT bias; per-head [rows, dk]
    k/v chunks are carved via fp16 PE transposes in groups of 4 psum slots
    with ONE wide fp16-2x copy per group (kt/vt column-padded so every
    transpose is a uniform [128,128]; garbage rows land in never-read kvc
    bytes).
  * Softmax: DVE reduce_max from T psum; the -max shift is applied back INTO
    the T psum by the PE (transpose negmax + rank-12 block-selector matmul),
    ACT exps straight out of psum, DVE sums + reciprocal, GPSIMD normalizes.
  * A-transposes are PAIRED: two adjacent 64-wide head-groups transposed as
    one [128,128] fp16 PE transpose straight into the mt layout; qt/mt are
    column-padded so every chunk uses an unclamped window and each (l,hf,jj)
    needs a single [128,256] copy (head-ordered overwrite heals the padding).
  * Transpose psum lives in per-use-alternating FULL-BANK tiles: psum
    accumulation-group tracking serializes a bank's new group against all
    pending reads of that bank, so slots sharing a bank lock-step.
  * The single output projection runs in three x-row ranges (head regions) so
    it interleaves with the tail of the softmax pipeline.

Sharding: flat reshape makes head h own flat rows [2048h, 2048(h+1)) of the
[B*24576, 64] flat view == rows [512c, 512(c+1)) of the [4096, 768] (B*S, D)
matrix for head-triple c. Core c gets x rows [512c, 512(c+1)) and heads
{3c, 3c+1, 3c+2} - fully local, no collectives.
"""

import numpy as np

import concourse.bass as bass
import concourse.mybir as mybir
import concourse.tile as tile
from concourse import bacc
from concourse.bass_utils import run_bass_kernel_spmd
from concourse.masks import make_identity

F32 = mybir.dt.float32
F16 = mybir.dt.float16

B, S, D = 2, 2048, 768
H, DK = 12, 64
SCALE = 0.125
NCORES = 8
SLOC = 512          # x rows per core
HLOC = 3            # heads per core
NCH = 24            # carve chunks per head (12 groups x 2)
KT_COLS = 640       # kt/vt padded so c=1 carve transposes are full [128,128]
QT_COLS = 640       # qt tiles padded so c=1 T-chunks never clamp
MT_COLS = 600       # mt tiles padded for the 256-wide paired copies (341+256)

ACT_ID = mybir.ActivationFunctionType.Identity
ACT_EXP = mybir.ActivationFunctionType.Exp


def _ceil_div(a, b):
    return -((-a) // b)


def _slabs():
    """Per (head l, group g): local x-row range [s_lo, s_hi) of the slab."""
    tab = {}
    for l in range(HLOC):
        tot = 0
        for g in range(12):
            s_lo = max(0, _ceil_div(2048 * l - g, 12))
            s_hi = min(SLOC, _ceil_div(2048 * (l + 1) - g, 12))
            tab[(l, g)] = (s_lo, s_hi)
            tot += s_hi - s_lo
        assert tot == 2048, tot
    for l in range(HLOC):
        for j in range(6):
            assert tab[(l, 2 * j)] == tab[(l, 2 * j + 1)]
    return tab


SLABS = _slabs()

# output projection x-row ranges (head regions; boundary cols go with the
# later head so each range only needs heads <= its index complete)
O_RANGES = [(0, 170), (170, 341), (341, SLOC)]


def build_nc():
    nc = bacc.Bacc()

    xT = nc.declare_dram_parameter("xT", [D, SLOC], F16, isOutput=False)
    wqT = nc.declare_dram_parameter("wqT", [D, D], F16, isOutput=False)
    wkT = nc.declare_dram_parameter("wkT", [D, D], F16, isOutput=False)
    wvT = nc.declare_dram_parameter("wvT", [D, D], F16, isOutput=False)
    wfoT = nc.declare_dram_parameter("wfoT", [D, D], F16, isOutput=False)
    # [:, i, j] = b_i[128j+p] for i in (q, k, v, fo') with bfo' = Wf@bo+bf
    bias_po = nc.declare_dram_parameter("bias_po", [128, 4, 6], F32, isOutput=False)
    # block selector: sel[ch, 64*ch':64*ch'+64] = (ch == ch')
    sel = nc.declare_dram_parameter("sel", [12, D], F16, isOutput=False)
    outT = nc.declare_dram_parameter("outT", [D, SLOC], F16, isOutput=True)

    with tile.TileContext(nc) as tc:
        with (
            tc.tile_pool(name="consts", bufs=1) as consts,
            tc.tile_pool(name="xw", bufs=1) as xwp,
            tc.tile_pool(name="ktv", bufs=1) as ktvp,
            tc.tile_pool(name="kvc", bufs=1) as kvcp,
            tc.tile_pool(name="qt", bufs=1) as qtp,
            tc.tile_pool(name="gsb", bufs=1) as gsbp,
            tc.tile_pool(name="mt", bufs=1) as mtp,
            tc.tile_pool(name="sm", bufs=4) as smp,
            tc.tile_pool(name="smx", bufs=4) as smxp,
            tc.tile_pool(name="outp", bufs=1) as outp,
            tc.tile_pool(name="pp", bufs=2, space="PSUM") as pp,
        ):
            # ---- constants ------------------------------------------------
            ident = consts.tile([128, 128], F16)
            make_identity(nc, ident)
            bias_sb = consts.tile([128, 4, 6], F32)
            nc.sync.dma_start(out=bias_sb, in_=bias_po[:, :, :])
            sel_sb = consts.tile([12, D], F16)
            nc.sync.dma_start(out=sel_sb, in_=sel[:, :])
            # pay the Exp act-table load during the DMA phase, not mid-softmax
            warm = consts.tile([1, 1], F32)
            nc.vector.memset(warm, 0.0)
            nc.scalar.activation(warm, warm, ACT_EXP)

            # ---- input DMAs ----------------------------------------------
            xT_sb = [xwp.tile([128, SLOC], F16, tag=f"x{k}", name=f"x{k}")
                     for k in range(6)]
            wqT_sb = [xwp.tile([128, D], F16, tag=f"wq{k}", name=f"wq{k}")
                      for k in range(6)]
            wkT_sb = [xwp.tile([128, D], F16, tag=f"wk{k}", name=f"wk{k}")
                      for k in range(6)]
            wvT_sb = [xwp.tile([128, D], F16, tag=f"wv{k}", name=f"wv{k}")
                      for k in range(6)]
            wfoT_sb = [xwp.tile([128, D], F16, tag=f"wfo{k}", name=f"wfo{k}")
                       for k in range(6)]
            for k in range(6):
                nc.sync.dma_start(out=xT_sb[k], in_=xT[128 * k:128 * (k + 1), :])
            for k in range(6):
                nc.gpsimd.dma_start(out=wkT_sb[k], in_=wkT[128 * k:128 * (k + 1), :])
            for k in range(6):
                nc.sync.dma_start(out=wvT_sb[k], in_=wvT[128 * k:128 * (k + 1), :])
            for k in range(6):
                nc.gpsimd.dma_start(out=wqT_sb[k], in_=wqT[128 * k:128 * (k + 1), :])
            for k in range(6):
                nc.gpsimd.dma_start(out=wfoT_sb[k],
                                    in_=wfoT[128 * k:128 * (k + 1), :])

            # ---- persistent tiles ----------------------------------------
            kt_sb = [ktvp.tile([128, KT_COLS], F16, tag=f"kt{j}", name=f"kt{j}")
                     for j in range(6)]
            vt_sb = [ktvp.tile([128, KT_COLS], F16, tag=f"vt{j}", name=f"vt{j}")
                     for j in range(6)]
            # packed per-head carve: k at [:, 0, ch, :], v at [:, 1, ch, :]
            # chunk index ch = 4j + 2c + h2
            kvc = [kvcp.tile([128, 2, NCH, DK], F16, tag=f"kvc{l}",
                             name=f"kvc{l}") for l in range(HLOC)]
            qt_sb = [qtp.tile([128, QT_COLS], F16, tag=f"qt{j}", name=f"qt{j}")
                     for j in range(6)]
            # zero-padded G parity variants: [:, 0, l, :] = [G_l; 0],
            # [:, 1, l, :] = [0; G_l]
            g_sb = gsbp.tile([128, 2, HLOC, DK], F16)
            nc.scalar.memzero(g_sb[:, :, :, :])
            mt_sb = [mtp.tile([128, MT_COLS], F16, tag=f"mt{j}", name=f"mt{j}")
                     for j in range(6)]
            out_sb = [outp.tile([128, SLOC], F16, tag=f"ou{j}", name=f"ou{j}")
                      for j in range(6)]
            # zero padding cols (read by unclamped c=1 chunks)
            for j in range(6):
                nc.vector.memset(qt_sb[j][:, SLOC:QT_COLS], 0.0)
                nc.vector.memset(kt_sb[j][:, SLOC:KT_COLS], 0.0)
                nc.vector.memset(vt_sb[j][:, SLOC:KT_COLS], 0.0)


            # ---- emission helpers ----------------------------------------
            def proj(bi, w_sb, dst, j):
                """Feature-major projection column block j with ACT bias."""
                ps = pp.tile([128, 512], F32, tag="pp")
                for k in range(6):
                    nc.tensor.matmul(
                        ps,
                        w_sb[k][:, 128 * j:128 * (j + 1)],
                        xT_sb[k],
                        start=(k == 0), stop=(k == 5),
                    )
                nc.scalar.activation(
                    dst, ps, ACT_ID, bias=bias_sb[:, bi, j:j + 1],
                )

            # per-batch softmax state
            tps_t = {}
            sm_t = {}
            nm_t = {}

            def _sm_shift(l, hf, nmT_ps):
                """PE: transpose negmax into nmT_ps (psum), copy to SBUF,
                then add -max into the T psum via a rank-12 block-selector
                matmul (closing writes into the already-stopped groups)."""
                tps = tps_t[(l, hf)]
                negmax = nm_t[(l, hf)]
                nc.tensor.transpose(nmT_ps, negmax, ident)
                nmT = smxp.tile([12, 128], F16, tag="nmT",
                                name=f"nmT{l}{hf}")
                nc.scalar.copy(nmT, nmT_ps)
                nc.tensor.matmul(tps[:, 0:8, :], nmT, sel_sb[:, 0:512],
                                 start=False, stop=False,
                                 skip_group_check=True)
                nc.tensor.matmul(tps[:, 8:12, :], nmT, sel_sb[:, 512:768],
                                 start=False, stop=False,
                                 skip_group_check=True)

            pools = {}

            def t_mms(l, hf):
                """T = q @ G chunks for batch (l, hf) -> psum [128, 12, 64]."""
                tps = pools["ppt"].tile([128, 12, DK], F32, tag="T",
                                        name=f"T{l}{hf}")
                tps_t[(l, hf)] = tps
                for jj in range(3):
                    J = 3 * hf + jj
                    for c in range(2):
                        for p in range(2):
                            g = 6 * hf + 2 * jj + p
                            s_lo, _ = SLABS[(l, g)]
                            col0 = s_lo + 128 * c
                            ch = 4 * jj + 2 * c + p
                            nc.tensor.matmul(
                                tps[:, ch, :],
                                qt_sb[J][:, col0:col0 + 128],
                                g_sb[:, p, l, :],
                                start=(ch == 0 or ch == 8),
                                stop=(ch == 7 or ch == 11),
                            )

            def sm_a(l, hf):
                """negmax of the T psum (DVE)."""
                tps = tps_t[(l, hf)]
                negmax = smxp.tile([128, 12], F16, tag="nm", name=f"nm{l}{hf}")
                with nc.allow_low_precision(reason="shift only needs ~ulp(max)"):
                    nc.vector.reduce_max(negmax, tps, axis=mybir.AxisListType.X,
                                         negate=True)
                nm_t[(l, hf)] = negmax

            def sm_add(l, hf):
                """Low-latency shift variant: DVE broadcast-add -> fp16."""
                tps = tps_t[(l, hf)]
                negmax = nm_t[(l, hf)]
                pre = smp.tile([128, 12, DK], F16, tag="A", name=f"P{l}{hf}")
                nm_b = bass.AP(tensor=negmax.tensor, offset=negmax.offset,
                               ap=[negmax.ap[0], negmax.ap[1], [0, DK]])
                nc.vector.tensor_add(pre, tps, nm_b)
                sm_t[(l, hf)] = pre
                tps_t[(l, hf)] = pre

            def sm_b(l, hf):
                """exp (ACT), sum+recip (DVE), normalize (GPSIMD)."""
                tps = tps_t[(l, hf)]
                av = smp.tile([128, 12, DK], F16, tag="A", name=f"A{l}{hf}")
                sm_t[(l, hf)] = av
                nc.scalar.activation(av, tps, ACT_EXP)
                s = smxp.tile([128, 12], F16, tag="sm", name=f"sm{l}{hf}")
                inv = smxp.tile([128, 12], F16, tag="inv", name=f"inv{l}{hf}")
                with nc.allow_low_precision(reason="exp sums in [1,64], fp16 ok"):
                    nc.vector.reduce_sum(s, av, axis=mybir.AxisListType.X)
                    nc.vector.reciprocal(inv, s)
                inv_b = bass.AP(tensor=inv.tensor, offset=inv.offset,
                                ap=[inv.ap[0], inv.ap[1], [0, DK]])
                nc.gpsimd.tensor_mul(av, av, inv_b)

            def o_range(r):
                r0, r1 = O_RANGES[r]
                ln = r1 - r0
                for j in range(6):
                    ps = pp.tile([128, 512], F32, tag="pp")
                    for k in range(6):
                        nc.tensor.matmul(
                            ps[:, 0:ln],
                            wfoT_sb[k][:, 128 * j:128 * (j + 1)],
                            mt_sb[k][:, r0:r1],
                            start=(k == 0), stop=(k == 5),
                        )
                    nc.scalar.activation(
                        out_sb[j][:, r0:r1], ps[:, 0:ln], ACT_ID,
                        bias=bias_sb[:, 3, j:j + 1],
                    )
                    if r == 2 or j % 2 == 0:
                        nc.sync.dma_start(
                            out=outT[128 * j:128 * (j + 1), r0:r1],
                            in_=out_sb[j][:, r0:r1])
                    else:
                        nc.gpsimd.dma_start(
                            out=outT[128 * j:128 * (j + 1), r0:r1],
                            in_=out_sb[j][:, r0:r1])

            # ---- phase 1: K/V projections, carve, G ----------------------
            with (
                tc.tile_pool(name="ppg", bufs=1, space="PSUM") as ppg,
                tc.tile_pool(name="pptr", bufs=1, space="PSUM") as pptr,
            ):
                gps = ppg.tile([DK, HLOC, DK], F32)
                # 4 transpose slots per bank, two alternating banks; each
                # group is consumed by ONE wide fp16-2x copy, so bank
                # group-WAR costs once per 4 slots and adjacent groups
                # overlap across banks
                trp4 = [pptr.tile([128, 4, 2, 128], F16, tag=f"trp4{i}",
                                  name=f"trp4{i}") for i in range(2)]

                def carve(l):
                    """Per-head packed [rows, dk] k/v chunks, 4-slot groups."""
                    for jp in range(3):
                        tr = trp4[(3 * l + jp) % 2]
                        for dj in range(2):
                            j = 2 * jp + dj
                            s_lo, _ = SLABS[(l, 2 * j)]
                            for c in range(2):
                                s0 = s_lo + 128 * c
                                slot = 2 * dj + c
                                nc.tensor.transpose(
                                    tr[:, slot, 0, :],
                                    kt_sb[j][:, s0:s0 + 128], ident)
                                nc.tensor.transpose(
                                    tr[:, slot, 1, :],
                                    vt_sb[j][:, s0:s0 + 128], ident)
                        # one copy: (kv, slot, h2, d) -> kvc chunks 8jp..8jp+8
                        srcap = bass.AP(
                            tensor=tr.tensor, offset=tr.offset,
                            ap=[tr.ap[0], [128, 2], [256, 4], [64, 2],
                                [1, 64]])
                        kt = kvc[l]
                        dstap = bass.AP(
                            tensor=kt.tensor,
                            offset=kt.offset + 8 * jp * DK,
                            ap=[kt.ap[0], [NCH * DK, 2], [2 * DK, 4],
                                [DK, 2], [1, DK]])
                        nc.vector.tensor_copy(dstap, srcap)

                def g_head(l):
                    pieces = []
                    for g in range(12):
                        s_lo, s_hi = SLABS[(l, g)]
                        L = s_hi - s_lo
                        j, h2 = g // 2, g % 2
                        pieces.append((4 * j + h2, min(128, L)))
                        if L > 128:
                            pieces.append((4 * j + 2 + h2, L - 128))
                    for i, (ch, kk) in enumerate(pieces):
                        nc.tensor.matmul(
                            gps[:, l, :],
                            kvc[l][0:kk, 0, ch, :],
                            kvc[l][0:kk, 1, ch, :],
                            start=(i == 0), stop=(i == len(pieces) - 1),
                        )
                    nc.vector.tensor_scalar_mul(
                        g_sb[0:DK, 0, l, :], gps[:, l, :], SCALE)
                    nc.vector.tensor_copy(
                        g_sb[DK:128, 1, l, :], g_sb[0:DK, 0, l, :])

                for j in range(6):
                    proj(1, wkT_sb, kt_sb[j][:, 0:SLOC], j)
                for j in range(6):
                    proj(2, wvT_sb, vt_sb[j][:, 0:SLOC], j)
                carve(0); g_head(0)
                proj(0, wqT_sb, qt_sb[0][:, 0:SLOC], 0)
                carve(1); g_head(1)
                proj(0, wqT_sb, qt_sb[1][:, 0:SLOC], 1)
                carve(2); g_head(2)
                proj(0, wqT_sb, qt_sb[2][:, 0:SLOC], 2)

            # ---- phase 2: T, softmax, A-transposes, output ---------------
            with (
                tc.tile_pool(name="ppt", bufs=2, space="PSUM") as ppt,
                tc.tile_pool(name="pptr2", bufs=1, space="PSUM") as pptr2,
            ):
                pools["ppt"] = ppt
                # two full-bank transpose tiles, alternated per use so a new
                # psum group never waits on the previous use's read
                trq = [pptr2.tile([128, 2, 128], F16, tag=f"trq{i}",
                                  name=f"trq{i}") for i in range(2)]
                trq_n = [0]

                def sm_shift(l, hf):
                    t = trq[trq_n[0] % 2]
                    trq_n[0] += 1
                    _sm_shift(l, hf, t[0:12, 0, :])

                def atr(l, hf):
                    """Paired transposes into mt (feature-major attn)."""
                    av = sm_t[(l, hf)]
                    for jj in range(3):
                        J = 3 * hf + jj
                        g = 6 * hf + 2 * jj
                        s_lo, _ = SLABS[(l, g)]
                        t = trq[trq_n[0] % 2]
                        trq_n[0] += 1
                        for c in range(2):
                            ch = 4 * jj + 2 * c
                            nc.tensor.transpose(
                                t[:, c, :], av[:, ch:ch + 2, :], ident)
                        if jj % 2 == 0:
                            nc.vector.tensor_copy(
                                mt_sb[J][:, s_lo:s_lo + 256], t[:, :, :])
                        else:
                            nc.scalar.copy(
                                mt_sb[J][:, s_lo:s_lo + 256], t[:, :, :])

                t_mms(0, 0); sm_a(0, 0)
                for j in range(3, 6):
                    proj(0, wqT_sb, qt_sb[j][:, 0:SLOC], j)
                sm_shift(0, 0)
                t_mms(1, 0); sm_a(1, 0); sm_b(0, 0)
                sm_shift(1, 0)
                t_mms(0, 1); sm_a(0, 1); sm_b(1, 0)
                sm_shift(0, 1)
                t_mms(1, 1); sm_a(1, 1); sm_b(0, 1)
                atr(0, 0)
                sm_shift(1, 1)
                t_mms(2, 0); sm_a(2, 0); sm_b(1, 1)
                atr(1, 0)
                sm_shift(2, 0)
                t_mms(2, 1); sm_a(2, 1); sm_b(2, 0)
                atr(0, 1)
                o_range(0)
                sm_shift(2, 1)
                sm_b(2, 1)
                atr(1, 1)
                o_range(1)
                atr(2, 0); atr(2, 1)
                o_range(2)

    nc.finalize()
    return nc


_NC_CACHE = None


def make_in_maps(x, Wq, bq, Wk, bk, Wv, bv, Wo, bo, Wf, bf):
    f32 = np.float32
    xf = np.asarray(x, f32).reshape(B * S, D)
    Wfo = np.asarray(Wf, f32) @ np.asarray(Wo, f32)
    bfo = np.asarray(Wf, f32) @ np.asarray(bo, f32) + np.asarray(bf, f32)
    shared = {
        "wqT": np.ascontiguousarray(np.asarray(Wq, f32).T).astype(np.float16),
        "wkT": np.ascontiguousarray(np.asarray(Wk, f32).T).astype(np.float16),
        "wvT": np.ascontiguousarray(np.asarray(Wv, f32).T).astype(np.float16),
        "wfoT": np.ascontiguousarray(Wfo.T).astype(np.float16),
        "bias_po": np.stack(
            [np.asarray(b, f32).reshape(6, 128).T
             for b in (bq, bk, bv, bfo)],
            axis=1,
        ).copy(),
        "sel": np.kron(np.eye(12, dtype=f32),
                       np.ones((1, DK), f32)).astype(np.float16),
    }
    in_maps = []
    for c in range(NCORES):
        m = dict(shared)
        m["xT"] = np.ascontiguousarray(
            xf[SLOC * c:SLOC * (c + 1), :].T).astype(np.float16)
        in_maps.append(m)
    return in_maps


def kernel(**inputs):
    global _NC_CACHE
    if _NC_CACHE is None:
        _NC_CACHE = build_nc()
    nc = _NC_CACHE
    in_maps = make_in_maps(**inputs)
    res = run_bass_kernel_spmd(nc, in_maps, list(range(NCORES)))
    out = np.empty((B * S, D), np.float32)
    for c in range(NCORES):
        out[SLOC * c:SLOC * (c + 1), :] = res.results[c]["outT"].T.astype(np.float32)
    return out.reshape(B, S, D)


# revision 3
# speedup vs baseline: 1.2215x; 1.2215x over previous
"""Trainium2 Bass kernel v2 for nn_Attention_18760417149505.

Reference (per problem):
  q/k/v = (x @ W.T + b).reshape(B, H, S, dk)      # flat reshape, NOT head-split
  scores = q @ k.T ; t = (scores*SCALE) @ v ; attn = softmax(t, axis=-1)
  out = ((attn.reshape(B,S,D) @ Wo.T + bo) @ Wf.T + bf)

Softmax comes AFTER both score matmuls, so per (batch, head) only the 64x64
Gram matrix G = SCALE * k.T @ v is needed:  t = q @ G.

v2 design (vs the fp32r baseline):
  * fp16 on the whole matmul path (1 cycle/row at ANY free size, 2-byte DMA).
    Empirically rel_err ~4.3e-3 vs the 2e-2 gate; bf16 fails (3.3e-2) because
    the pre-softmax logits (std ~45) make softmax near-argmax.
  * Wf@Wo folded into one weight on the host: one 768x768 projection less.
  * Q/K/V all feature-major with per-partition ACT bias; per-head [rows, dk]
    k/v chunks are carved via fp16 PE transposes in groups of 4 psum slots
    with ONE wide fp16-2x copy per group (kt/vt column-padded so every
    transpose is a uniform [128,128]; garbage rows land in never-read kvc
    bytes).
  * Softmax pipeline (six (head, group-half) batches, software-pipelined
    across engines): DVE reduce_max from the T psum; the -max shift is
    applied back INTO the T psum by the PE (transpose negmax + rank-12
    block-selector matmul accumulating into the closed groups); ACT exps
    straight out of psum; DVE sums + reciprocal; GPSIMD normalizes
    (SBUF-only engine - it has no PSUM port).
  * A-transposes are PAIRED: two adjacent 64-wide head-groups transposed as
    one [128,128] fp16 PE transpose straight into the mt layout; qt/mt are
    column-padded so every chunk uses an unclamped window and each (l,hf,jj)
    needs a single [128,256] copy (head-ordered overwrite heals the padding).
  * Transpose psum lives in per-use-alternating FULL-BANK tiles: psum
    accumulation-group tracking serializes a bank's new group against all
    pending reads of that bank, so slots sharing a bank lock-step.
  * The single output projection runs in three x-row ranges (head regions) so
    it starts as each head's attention completes and interleaves with the
    tail of the softmax pipeline; outputs stream out per (range, j).
  * Cost-model profile (CoreSim): 57.3us vs the fp32r baseline's 98.8us;
    PE busy 42.7us (the fp16 matmul work itself), rel_err 4.1e-3.

Sharding: flat reshape makes head h own flat rows [2048h, 2048(h+1)) of the
[B*24576, 64] flat view == rows [512c, 512(c+1)) of the [4096, 768] (B*S, D)
matrix for head-triple c. Core c gets x rows [512c, 512(c+1)) and heads
{3c, 3c+1, 3c+2} - fully local, no collectives.
"""

import numpy as np

import concourse.bass as bass
import concourse.mybir as mybir
import concourse.tile as tile
from concourse import bacc
from concourse.bass_utils import run_bass_kernel_spmd
from concourse.masks import make_identity

F32 = mybir.dt.float32
F16 = mybir.dt.float16

B, S, D = 2, 2048, 768
H, DK = 12, 64
SCALE = 0.125
NCORES = 8
SLOC = 512          # x rows per core
HLOC = 3            # heads per core
NCH = 24            # carve chunks per head (12 groups x 2)
KT_COLS = 640       # kt/vt padded so c=1 carve transposes are full [128,128]
QT_COLS = 640       # qt tiles padded so c=1 T-chunks never clamp
MT_COLS = 600       # mt tiles padded for the 256-wide paired copies (341+256)

ACT_ID = mybir.ActivationFunctionType.Identity
ACT_EXP = mybir.ActivationFunctionType.Exp


def _ceil_div(a, b):
    return -((-a) // b)


def _slabs():
    """Per (head l, group g): local x-row range [s_lo, s_hi) of the slab."""
    tab = {}
    for l in range(HLOC):
        tot = 0
        for g in range(12):
            s_lo = max(0, _ceil_div(2048 * l - g, 12))
            s_hi = min(SLOC, _ceil_div(2048 * (l + 1) - g, 12))
            tab[(l, g)] = (s_lo, s_hi)
            tot += s_hi - s_lo
        assert tot == 2048, tot
    for l in range(HLOC):
        for j in range(6):
            assert tab[(l, 2 * j)] == tab[(l, 2 * j + 1)]
    return tab


SLABS = _slabs()

# output projection x-row ranges (head regions; boundary cols go with the
# later head so each range only needs heads <= its index complete)
O_RANGES = [(0, 170), (170, 341), (341, SLOC)]


def build_nc():
    nc = bacc.Bacc()

    xT = nc.declare_dram_parameter("xT", [D, SLOC], F16, isOutput=False)
    wqT = nc.declare_dram_parameter("wqT", [D, D], F16, isOutput=False)
    wkT = nc.declare_dram_parameter("wkT", [D, D], F16, isOutput=False)
    wvT = nc.declare_dram_parameter("wvT", [D, D], F16, isOutput=False)
    wfoT = nc.declare_dram_parameter("wfoT", [D, D], F16, isOutput=False)
    # [:, i, j] = b_i[128j+p] for i in (q, k, v, fo') with bfo' = Wf@bo+bf
    bias_po = nc.declare_dram_parameter("bias_po", [128, 4, 6], F32, isOutput=False)
    # block selector: sel[ch, 64*ch':64*ch'+64] = (ch == ch')
    sel = nc.declare_dram_parameter("sel", [12, D], F16, isOutput=False)
    outT = nc.declare_dram_parameter("outT", [D, SLOC], F16, isOutput=True)

    with tile.TileContext(nc) as tc:
        with (
            tc.tile_pool(name="consts", bufs=1) as consts,
            tc.tile_pool(name="xw", bufs=1) as xwp,
            tc.tile_pool(name="ktv", bufs=1) as ktvp,
            tc.tile_pool(name="kvc", bufs=1) as kvcp,
            tc.tile_pool(name="qt", bufs=1) as qtp,
            tc.tile_pool(name="gsb", bufs=1) as gsbp,
            tc.tile_pool(name="mt", bufs=1) as mtp,
            tc.tile_pool(name="sm", bufs=4) as smp,
            tc.tile_pool(name="smx", bufs=4) as smxp,
            tc.tile_pool(name="outp", bufs=1) as outp,
            tc.tile_pool(name="pp", bufs=2, space="PSUM") as pp,
        ):
            # ---- constants ------------------------------------------------
            ident = consts.tile([128, 128], F16)
            make_identity(nc, ident)
            bias_sb = consts.tile([128, 4, 6], F32)
            nc.sync.dma_start(out=bias_sb, in_=bias_po[:, :, :])
            sel_sb = consts.tile([12, D], F16)
            nc.sync.dma_start(out=sel_sb, in_=sel[:, :])
            # pay the Exp act-table load during the DMA phase, not mid-softmax
            warm = consts.tile([1, 1], F32)
            nc.vector.memset(warm, 0.0)
            nc.scalar.activation(warm, warm, ACT_EXP)

            # ---- input DMAs ----------------------------------------------
            xT_sb = [xwp.tile([128, SLOC], F16, tag=f"x{k}", name=f"x{k}")
                     for k in range(6)]
            wqT_sb = [xwp.tile([128, D], F16, tag=f"wq{k}", name=f"wq{k}")
                      for k in range(6)]
            wkT_sb = [xwp.tile([128, D], F16, tag=f"wk{k}", name=f"wk{k}")
                      for k in range(6)]
            wvT_sb = [xwp.tile([128, D], F16, tag=f"wv{k}", name=f"wv{k}")
                      for k in range(6)]
            wfoT_sb = [xwp.tile([128, D], F16, tag=f"wfo{k}", name=f"wfo{k}")
                       for k in range(6)]
            for k in range(6):
                nc.sync.dma_start(out=xT_sb[k], in_=xT[128 * k:128 * (k + 1), :])
            for k in range(6):
                nc.gpsimd.dma_start(out=wkT_sb[k], in_=wkT[128 * k:128 * (k + 1), :])
            for k in range(6):
                nc.sync.dma_start(out=wvT_sb[k], in_=wvT[128 * k:128 * (k + 1), :])
            for k in range(6):
                nc.gpsimd.dma_start(out=wqT_sb[k], in_=wqT[128 * k:128 * (k + 1), :])
            for k in range(6):
                nc.gpsimd.dma_start(out=wfoT_sb[k],
                                    in_=wfoT[128 * k:128 * (k + 1), :])

            # ---- persistent tiles ----------------------------------------
            kt_sb = [ktvp.tile([128, KT_COLS], F16, tag=f"kt{j}", name=f"kt{j}")
                     for j in range(6)]
            vt_sb = [ktvp.tile([128, KT_COLS], F16, tag=f"vt{j}", name=f"vt{j}")
                     for j in range(6)]
            # packed per-head carve: k at [:, 0, ch, :], v at [:, 1, ch, :]
            # chunk index ch = 4j + 2c + h2
            kvc = [kvcp.tile([128, 2, NCH, DK], F16, tag=f"kvc{l}",
                             name=f"kvc{l}") for l in range(HLOC)]
            qt_sb = [qtp.tile([128, QT_COLS], F16, tag=f"qt{j}", name=f"qt{j}")
                     for j in range(6)]
            # zero-padded G parity variants: [:, 0, l, :] = [G_l; 0],
            # [:, 1, l, :] = [0; G_l]
            g_sb = gsbp.tile([128, 2, HLOC, DK], F16)
            nc.scalar.memzero(g_sb[:, :, :, :])
            mt_sb = [mtp.tile([128, MT_COLS], F16, tag=f"mt{j}", name=f"mt{j}")
                     for j in range(6)]
            out_sb = [outp.tile([128, SLOC], F16, tag=f"ou{j}", name=f"ou{j}")
                      for j in range(6)]
            # zero padding cols (read by unclamped c=1 chunks)
            for j in range(6):
                nc.vector.memset(qt_sb[j][:, SLOC:QT_COLS], 0.0)
                nc.vector.memset(kt_sb[j][:, SLOC:KT_COLS], 0.0)
                nc.vector.memset(vt_sb[j][:, SLOC:KT_COLS], 0.0)


            # ---- emission helpers ----------------------------------------
            def proj(bi, w_sb, dst, j):
                """Feature-major projection column block j with ACT bias."""
                ps = pp.tile([128, 512], F32, tag="pp")
                for k in range(6):
                    nc.tensor.matmul(
                        ps,
                        w_sb[k][:, 128 * j:128 * (j + 1)],
                        xT_sb[k],
                        start=(k == 0), stop=(k == 5),
                    )
                nc.scalar.activation(
                    dst, ps, ACT_ID, bias=bias_sb[:, bi, j:j + 1],
                )

            # per-batch softmax state
            tps_t = {}
            sm_t = {}
            nm_t = {}

            def _sm_shift(l, hf, nmT_ps):
                """PE: transpose negmax into nmT_ps (psum), copy to SBUF,
                then add -max into the T psum via a rank-12 block-selector
                matmul (closing writes into the already-stopped groups)."""
                tps = tps_t[(l, hf)]
                negmax = nm_t[(l, hf)]
                nc.tensor.transpose(nmT_ps, negmax, ident)
                nmT = smxp.tile([12, 128], F16, tag="nmT",
                                name=f"nmT{l}{hf}")
                nc.scalar.copy(nmT, nmT_ps)
                nc.tensor.matmul(tps[:, 0:8, :], nmT, sel_sb[:, 0:512],
                                 start=False, stop=False,
                                 skip_group_check=True)
                nc.tensor.matmul(tps[:, 8:12, :], nmT, sel_sb[:, 512:768],
                                 start=False, stop=False,
                                 skip_group_check=True)

            pools = {}

            def t_mms(l, hf):
                """T = q @ G chunks for batch (l, hf) -> psum [128, 12, 64]."""
                tps = pools["ppt"].tile([128, 12, DK], F32, tag="T",
                                        name=f"T{l}{hf}")
                tps_t[(l, hf)] = tps
                for jj in range(3):
                    J = 3 * hf + jj
                    for c in range(2):
                        for p in range(2):
                            g = 6 * hf + 2 * jj + p
                            s_lo, _ = SLABS[(l, g)]
                            col0 = s_lo + 128 * c
                            ch = 4 * jj + 2 * c + p
                            nc.tensor.matmul(
                                tps[:, ch, :],
                                qt_sb[J][:, col0:col0 + 128],
                                g_sb[:, p, l, :],
                                start=(ch == 0 or ch == 8),
                                stop=(ch == 7 or ch == 11),
                            )

            def sm_a(l, hf):
                """negmax of the T psum (DVE)."""
                tps = tps_t[(l, hf)]
                negmax = smxp.tile([128, 12], F16, tag="nm", name=f"nm{l}{hf}")
                with nc.allow_low_precision(reason="shift only needs ~ulp(max)"):
                    nc.vector.reduce_max(negmax, tps, axis=mybir.AxisListType.X,
                                         negate=True)
                nm_t[(l, hf)] = negmax

            def sm_add(l, hf):
                """Low-latency shift variant: DVE broadcast-add -> fp16."""
                tps = tps_t[(l, hf)]
                negmax = nm_t[(l, hf)]
                pre = smp.tile([128, 12, DK], F16, tag="A", name=f"P{l}{hf}")
                nm_b = bass.AP(tensor=negmax.tensor, offset=negmax.offset,
                               ap=[negmax.ap[0], negmax.ap[1], [0, DK]])
                nc.vector.tensor_add(pre, tps, nm_b)
                sm_t[(l, hf)] = pre
                tps_t[(l, hf)] = pre

            def sm_b(l, hf):
                """exp (ACT), sum+recip (DVE), normalize (GPSIMD)."""
                tps = tps_t[(l, hf)]
                av = smp.tile([128, 12, DK], F16, tag="A", name=f"A{l}{hf}")
                sm_t[(l, hf)] = av
                nc.scalar.activation(av, tps, ACT_EXP)
                s = smxp.tile([128, 12], F16, tag="sm", name=f"sm{l}{hf}")
                inv = smxp.tile([128, 12], F16, tag="inv", name=f"inv{l}{hf}")
                with nc.allow_low_precision(reason="exp sums in [1,64], fp16 ok"):
                    nc.vector.reduce_sum(s, av, axis=mybir.AxisListType.X)
                    nc.vector.reciprocal(inv, s)
                inv_b = bass.AP(tensor=inv.tensor, offset=inv.offset,
                                ap=[inv.ap[0], inv.ap[1], [0, DK]])
                nc.gpsimd.tensor_mul(av, av, inv_b)

            def o_range(r):
                r0, r1 = O_RANGES[r]
                ln = r1 - r0
                for j in range(6):
                    ps = pp.tile([128, 512], F32, tag="pp")
                    for k in range(6):
                        nc.tensor.matmul(
                            ps[:, 0:ln],
                            wfoT_sb[k][:, 128 * j:128 * (j + 1)],
                            mt_sb[k][:, r0:r1],
                            start=(k == 0), stop=(k == 5),
                        )
                    nc.scalar.activation(
                        out_sb[j][:, r0:r1], ps[:, 0:ln], ACT_ID,
                        bias=bias_sb[:, 3, j:j + 1],
                    )
                    if r == 2 or j % 2 == 0:
                        nc.sync.dma_start(
                            out=outT[128 * j:128 * (j + 1), r0:r1],
                            in_=out_sb[j][:, r0:r1])
                    else:
                        nc.gpsimd.dma_start(
                            out=outT[128 * j:128 * (j + 1), r0:r1],
                            in_=out_sb[j][:, r0:r1])

            # ---- phase 1: K/V projections, carve, G ----------------------
            with (
                tc.tile_pool(name="ppg", bufs=1, space="PSUM") as ppg,
                tc.tile_pool(name="pptr", bufs=1, space="PSUM") as pptr,
            ):
                gps = ppg.tile([DK, HLOC, DK], F32)
                # 4 transpose slots per bank, two alternating banks; each
                # group is consumed by ONE wide fp16-2x copy, so bank
                # group-WAR costs once per 4 slots and adjacent groups
                # overlap across banks
                trp4 = [pptr.tile([128, 4, 2, 128], F16, tag=f"trp4{i}",
                                  name=f"trp4{i}") for i in range(2)]

                def carve(l):
                    """Per-head packed [rows, dk] k/v chunks, 4-slot groups."""
                    for jp in range(3):
                        tr = trp4[(3 * l + jp) % 2]
                        for dj in range(2):
                            j = 2 * jp + dj
                            s_lo, _ = SLABS[(l, 2 * j)]
                            for c in range(2):
                                s0 = s_lo + 128 * c
                                slot = 2 * dj + c
                                nc.tensor.transpose(
                                    tr[:, slot, 0, :],
                                    kt_sb[j][:, s0:s0 + 128], ident)
                                nc.tensor.transpose(
                                    tr[:, slot, 1, :],
                                    vt_sb[j][:, s0:s0 + 128], ident)
                        # one copy: (kv, slot, h2, d) -> kvc chunks 8jp..8jp+8
                        srcap = bass.AP(
                            tensor=tr.tensor, offset=tr.offset,
                            ap=[tr.ap[0], [128, 2], [256, 4], [64, 2],
                                [1, 64]])
                        kt = kvc[l]
                        dstap = bass.AP(
                            tensor=kt.tensor,
                            offset=kt.offset + 8 * jp * DK,
                            ap=[kt.ap[0], [NCH * DK, 2], [2 * DK, 4],
                                [DK, 2], [1, DK]])
                        nc.vector.tensor_copy(dstap, srcap)

                def g_head(l):
                    pieces = []
                    for g in range(12):
                        s_lo, s_hi = SLABS[(l, g)]
                        L = s_hi - s_lo
                        j, h2 = g // 2, g % 2
                        pieces.append((4 * j + h2, min(128, L)))
                        if L > 128:
                            pieces.append((4 * j + 2 + h2, L - 128))
                    for i, (ch, kk) in enumerate(pieces):
                        nc.tensor.matmul(
                            gps[:, l, :],
                            kvc[l][0:kk, 0, ch, :],
                            kvc[l][0:kk, 1, ch, :],
                            start=(i == 0), stop=(i == len(pieces) - 1),
                        )
                    nc.vector.tensor_scalar_mul(
                        g_sb[0:DK, 0, l, :], gps[:, l, :], SCALE)
                    nc.vector.tensor_copy(
                        g_sb[DK:128, 1, l, :], g_sb[0:DK, 0, l, :])

                for j in range(6):
                    proj(1, wkT_sb, kt_sb[j][:, 0:SLOC], j)
                for j in range(6):
                    proj(2, wvT_sb, vt_sb[j][:, 0:SLOC], j)
                carve(0); g_head(0)
                proj(0, wqT_sb, qt_sb[0][:, 0:SLOC], 0)
                carve(1); g_head(1)
                proj(0, wqT_sb, qt_sb[1][:, 0:SLOC], 1)
                carve(2); g_head(2)
                proj(0, wqT_sb, qt_sb[2][:, 0:SLOC], 2)

            # ---- phase 2: T, softmax, A-transposes, output ---------------
            with (
                tc.tile_pool(name="ppt", bufs=2, space="PSUM") as ppt,
                tc.tile_pool(name="pptr2", bufs=1, space="PSUM") as pptr2,
            ):
                pools["ppt"] = ppt
                # two full-bank transpose tiles, alternated per use so a new
                # psum group never waits on the previous use's read
                trq = [pptr2.tile([128, 2, 128], F16, tag=f"trq{i}",
                                  name=f"trq{i}") for i in range(2)]
                trq_n = [0]

                def sm_shift(l, hf):
                    t = trq[trq_n[0] % 2]
                    trq_n[0] += 1
                    _sm_shift(l, hf, t[0:12, 0, :])

                def atr(l, hf):
                    """Paired transposes into mt (feature-major attn)."""
                    av = sm_t[(l, hf)]
                    for jj in range(3):
                        J = 3 * hf + jj
                        g = 6 * hf + 2 * jj
                        s_lo, _ = SLABS[(l, g)]
                        t = trq[trq_n[0] % 2]
                        trq_n[0] += 1
                        for c in range(2):
                            ch = 4 * jj + 2 * c
                            nc.tensor.transpose(
                                t[:, c, :], av[:, ch:ch + 2, :], ident)
                        if jj % 2 == 0:
                            nc.vector.tensor_copy(
                                mt_sb[J][:, s_lo:s_lo + 256], t[:, :, :])
                        else:
                            nc.scalar.copy(
                                mt_sb[J][:, s_lo:s_lo + 256], t[:, :, :])

                t_mms(0, 0); sm_a(0, 0)
                for j in range(3, 6):
                    proj(0, wqT_sb, qt_sb[j][:, 0:SLOC], j)
                sm_shift(0, 0)
                t_mms(1, 0); sm_a(1, 0); sm_b(0, 0)
                sm_shift(1, 0)
                t_mms(0, 1); sm_a(0, 1); sm_b(1, 0)
                sm_shift(0, 1)
                t_mms(1, 1); sm_a(1, 1); sm_b(0, 1)
                atr(0, 0)
                sm_shift(1, 1)
                t_mms(2, 0); sm_a(2, 0); sm_b(1, 1)
                atr(1, 0)
                sm_shift(2, 0)
                t_mms(2, 1); sm_a(2, 1); sm_b(2, 0)
                atr(0, 1)
                o_range(0)
                sm_shift(2, 1)
                sm_b(2, 1)
                atr(1, 1)
                o_range(1)
                atr(2, 0); atr(2, 1)
                o_range(2)

    nc.finalize()
    return nc


_NC_CACHE = None


def make_in_maps(x, Wq, bq, Wk, bk, Wv, bv, Wo, bo, Wf, bf):
    f32 = np.float32
    xf = np.asarray(x, f32).reshape(B * S, D)
    Wfo = np.asarray(Wf, f32) @ np.asarray(Wo, f32)
    bfo = np.asarray(Wf, f32) @ np.asarray(bo, f32) + np.asarray(bf, f32)
    shared = {
        "wqT": np.ascontiguousarray(np.asarray(Wq, f32).T).astype(np.float16),
        "wkT": np.ascontiguousarray(np.asarray(Wk, f32).T).astype(np.float16),
        "wvT": np.ascontiguousarray(np.asarray(Wv, f32).T).astype(np.float16),
        "wfoT": np.ascontiguousarray(Wfo.T).astype(np.float16),
        "bias_po": np.stack(
            [np.asarray(b, f32).reshape(6, 128).T
             for b in (bq, bk, bv, bfo)],
            axis=1,
        ).copy(),
        "sel": np.kron(np.eye(12, dtype=f32),
                       np.ones((1, DK), f32)).astype(np.float16),
    }
    in_maps = []
    for c in range(NCORES):
        m = dict(shared)
        m["xT"] = np.ascontiguousarray(
            xf[SLOC * c:SLOC * (c + 1), :].T).astype(np.float16)
        in_maps.append(m)
    return in_maps


def kernel(**inputs):
    global _NC_CACHE
    if _NC_CACHE is None:
        _NC_CACHE = build_nc()
    nc = _NC_CACHE
    in_maps = make_in_maps(**inputs)
    res = run_bass_kernel_spmd(nc, in_maps, list(range(NCORES)))
    out = np.empty((B * S, D), np.float32)
    for c in range(NCORES):
        out[SLOC * c:SLOC * (c + 1), :] = res.results[c]["outT"].T.astype(np.float32)
    return out.reshape(B, S, D)
